# revision 21
# baseline (speedup 1.0000x reference)
"""Trainium2 Bass kernel for nn_CosSimConv2D (v6 - fp8 DoubleRow corrections).

Math (per sample b):
  s    = im2col3x3(x) @ w_hat           where w_hat = w / (||w||_col + qv)
  out  = sign(s) * exp(a_u/2 * (ln(s^2) - ln(box)))
  box  = 3x3 box-filter of per-pixel sum(x^2)  (= ||im2col row||^2)
  a    = softmax(p)

GEMM precision: s = (xh+xl)@wh + xh@wl with x/w split hi+lo in bf16.
  - main term: 9 taps of [xh;xl] @ [wh;wh] (bf16, 128-contraction)
  - correction xh@wl: 8 taps in fp8(e4m3) DoubleRow matmuls (4 taps per
    matmul: 2 via partition halves x 2 via the k-pair axis), 1 tap in
    bf16. Everything is scaled 2^13 so the fp8 wl operand is O(1); the
    scale cancels in the epilogue via ln(box * 2^26) (activation scale).
  Measured (numpy, all 8 samples): rel l2 0.0079 vs the fp32 reference.
Data-parallel over batch: core b computes sample b.

Structure (v2 baseline, best-measured): host pre-builds the padded
transposed bf16 image [xh;xl] and the two fp8 tap-shifted plane-pair
images, so slab DMAs land ready-to-matmul (no on-device transposes or
shift copies). 8-row output tiles (N=1024 across a 2-bank PSUM tile,
two N=512 matmuls per stationary weight). The s2/box/ln(box) chain
runs 2-3 pairs ahead of the GEMM that consumes it; its w-direction
3-sum and (w,rows)->(rows,w) transpose fuse into one tiny matmul
bxT = timg^T @ band. GpSimd does only partition_broadcast; the box
flat DMA rides the scalar queue (never dams image issues).
"""

import sys

sys.path.insert(0, "/opt/trn_rl_repo")

import numpy as np
import ml_dtypes

import concourse.bass as bass
import concourse.mybir as mybir
import concourse.tile as tile
from concourse import bacc
from concourse.bass_utils import run_bass_kernel_spmd
from concourse.masks import make_identity

BF16 = mybir.dt.bfloat16
F32 = mybir.dt.float32
FP8 = mybir.dt.float8e4
U32 = mybir.dt.uint32
AF = mybir.ActivationFunctionType

B, H, W, C, UNITS = 8, 128, 128, 64, 128
HW = H * W  # 16384
HP, WP = H + 2, W + 2  # 130x130 padded image
SLAB = 16  # image rows per streaming slab
NSLAB = H // SLAB  # 8
PROWS = 8  # image rows per output pair-tile -> N = 1024
NPAIR = H // PROWS  # 16
NPIX = PROWS * W  # 1024
RING = 8  # lbc ring depth in pair-slots
SC = 2.0**13  # global weight scale (cancelled via ln(box*2^26))

_CACHE = {}


def _build():
    nc = bacc.Bacc("TRN2", target_bir_lowering=False, debug=False)

    img1_d = nc.dram_tensor("img1", [128, HP * WP], BF16, kind="ExternalInput")
    fa_d = nc.dram_tensor("fa", [128, 2 * H * W], FP8, kind="ExternalInput")
    fb_d = nc.dram_tensor("fb", [128, 2 * H * W], FP8, kind="ExternalInput")
    xs2_d = nc.dram_tensor("xs2", [128, H, C], BF16, kind="ExternalInput")
    wt13_d = nc.dram_tensor("wt13", [9, 128, UNITS], BF16, kind="ExternalInput")
    wsl2_d = nc.dram_tensor("wsl2", [64, UNITS], BF16, kind="ExternalInput")
    wa_d = nc.dram_tensor("wa", [128, 2, UNITS], FP8, kind="ExternalInput")
    wb_d = nc.dram_tensor("wb", [128, 2, UNITS], FP8, kind="ExternalInput")
    a2_d = nc.dram_tensor("a2", [128, 1], F32, kind="ExternalInput")
    band_d = nc.dram_tensor("band", [128, 128], BF16, kind="ExternalInput")
    out_d = nc.dram_tensor("out", [128, HW], F32, kind="ExternalOutput")

    with tile.TileContext(nc) as tc:
        with (
            tc.tile_pool(name="const", bufs=1) as constp,
            tc.tile_pool(name="big", bufs=1) as bigp,
            tc.tile_pool(name="xs2p", bufs=4) as xs2p,
            tc.tile_pool(name="box", bufs=2) as boxp,
            tc.tile_pool(name="epi", bufs=2) as epip,
            tc.tile_pool(name="pmm", bufs=3, space="PSUM") as pmmp,
            tc.tile_pool(name="pmisc", bufs=2, space="PSUM") as pmiscp,
        ):
            # ---- constants (front of the queues; all small) ----
            wt13 = constp.tile([128, 9, UNITS], BF16, tag="wt13")
            nc.sync.dma_start(out=wt13, in_=wt13_d.ap().rearrange("t k u -> k t u"))
            wsl2 = constp.tile([64, UNITS], BF16, tag="wsl2")
            nc.scalar.dma_start(out=wsl2, in_=wsl2_d[:, :])
            wa = constp.tile([128, 2, UNITS], FP8, tag="wa")
            nc.scalar.dma_start(out=wa, in_=wa_d[:, :, :])
            wb = constp.tile([128, 2, UNITS], FP8, tag="wb")
            nc.scalar.dma_start(out=wb, in_=wb_d[:, :, :])
            a2 = constp.tile([128, 1], F32, tag="a2")
            nc.scalar.dma_start(out=a2, in_=a2_d[:, :])
            band = constp.tile([128, 128], BF16, tag="band")
            nc.scalar.dma_start(out=band, in_=band_d[:, :])
            ident = constp.tile([128, 128], BF16, tag="ident")
            make_identity(nc, ident)

            # ---- big persistent buffers ----
            a1 = bigp.tile([128, HP * WP], BF16, tag="a1")
            fa = bigp.tile([128, 2, H, W], FP8, tag="fa")
            fb = bigp.tile([128, 2, H, W], FP8, tag="fb")
            a1v = a1.rearrange("p (hp wp) -> p hp wp", wp=WP)
            lbc = bigp.tile([128, RING * NPIX], BF16, tag="lbc")  # ln(box) ring
            s2p = bigp.tile([128, HP], BF16, tag="s2p")  # (w, padded h) sum x^2

            # ---- input slab DMAs ----
            img1v = img1_d.ap().rearrange("p (hp wp) -> p hp wp", wp=WP)
            fav = fa_d.ap().rearrange("p (j h w) -> p j h w", j=2, w=W)
            fbv = fb_d.ap().rearrange("p (j h w) -> p j h w", j=2, w=W)
            xs2_tiles = {}

            def issue_dma(s):
                if s >= NSLAB:
                    return
                r0 = 1 + s * SLAB
                h0 = s * SLAB
                nc.sync.dma_start(out=a1v[:, r0 : r0 + SLAB, :], in_=img1v[:, r0 : r0 + SLAB, :])
                nc.sync.dma_start(out=fa[:, :, h0 : h0 + SLAB, :], in_=fav[:, :, h0 : h0 + SLAB, :])
                nc.sync.dma_start(out=fb[:, :, h0 : h0 + SLAB, :], in_=fbv[:, :, h0 : h0 + SLAB, :])
                t = xs2p.tile([128, SLAB, C], BF16, tag="xs2")
                nc.scalar.dma_start(out=t, in_=xs2_d[:, s * SLAB : (s + 1) * SLAB, :])
                xs2_tiles[s] = t

            issue_dma(0)
            issue_dma(1)
            issue_dma(2)

            # PE warmup: real matmuls tick the HAM activity monitor
            # (transposes do not), so the array is at K=8/8 by the time
            # the first GEMM's data lands.
            wrm = pmiscp.tile([128, 128], F32, tag="misc")
            for _ in range(24):
                nc.tensor.matmul(wrm, ident, ident, start=True, stop=True)

            # Preload act table set 6 (natural_log_exp_and_others):
            # Square, Sign, Ln, Exp -> no ACT_TABLE_LOAD mid-kernel.
            nc.scalar.add_instruction(
                mybir.InstLoadActFuncSet(
                    name=nc.get_next_instruction_name(),
                    act_func_set_id=6,
                    ins=[],
                    outs=[],
                )
            )

            # zero the hp borders the DMAs never write
            nc.vector.memset(a1v[:, 0, :], 0.0)
            nc.vector.memset(a1v[:, HP - 1, :], 0.0)
            nc.vector.memset(s2p[:, 0:1], 0.0)
            nc.vector.memset(s2p[:, HP - 1 : HP], 0.0)

            def s2_slab(s):
                """sum-of-squares column of the norm image for slab s (hi only:
                the missing 2*hi*lo term is ~2^-9 relative on s2, ~a*5e-4 out)."""
                t = xs2_tiles.pop(s)
                xsq = xs2p.tile([128, SLAB, C], BF16, tag="xsq")
                nc.scalar.activation(out=xsq, in_=t, func=AF.Square)
                with nc.allow_low_precision(reason="s2 bf16 ~5e-4 rel; out err ~a*2.5e-4"):
                    nc.vector.tensor_reduce(
                        out=s2p[:, 1 + s * SLAB : 1 + (s + 1) * SLAB],
                        in_=xsq,
                        axis=mybir.AxisListType.X,
                        op=mybir.AluOpType.add,
                    )

            def box_pair(p):
                """ln(box * 2^26) for output pair p (rows 8p..8p+7) -> lbc ring.

                bxT = timg^T @ band does the w-direction 3-sum AND the
                (w,rows)->(rows,w) transpose in one tiny matmul; the 2^26
                (= SC^2) rides the Ln's free affine scale."""
                r0 = 8 * p
                timg = boxp.tile([128, PROWS], BF16, tag="timg")
                nc.vector.tensor_tensor(
                    out=timg,
                    in0=s2p[:, r0 : r0 + PROWS],
                    in1=s2p[:, r0 + 1 : r0 + 1 + PROWS],
                    op=mybir.AluOpType.add,
                )
                nc.vector.tensor_tensor(
                    out=timg,
                    in0=timg,
                    in1=s2p[:, r0 + 2 : r0 + 2 + PROWS],
                    op=mybir.AluOpType.add,
                )
                bxt = pmiscp.tile([PROWS, 128], F32, tag="misc")
                nc.tensor.matmul(bxt, timg, band, start=True, stop=True)
                lrow = boxp.tile([PROWS, 128], BF16, tag="lrow")
                nc.scalar.activation(out=lrow, in_=bxt, func=AF.Ln, scale=SC * SC)
                # flat rides the scalar queue: its wait (lrow, same engine,
                # just above) is satisfied by construction -> never dams
                # the sync queue's image issues behind the box chain.
                flat = boxp.tile([1, NPIX], BF16, tag="flat")
                nc.scalar.dma_start(
                    out=flat.rearrange("o (h w) -> o h w", w=W),
                    in_=lrow,
                )
                slot = p % RING
                nc.gpsimd.partition_broadcast(
                    lbc[:, slot * NPIX : (slot + 1) * NPIX], flat[:, 0:NPIX]
                )

            # ---- GEMM + epilogue per 8-row output pair-tile ----
            def emit_pair_gemm(p):
                hh = 8 * p
                ps = pmmp.tile([128, 2, 512], F32, tag="ps")
                # 12 stationary weights x two N=512 matmuls (rows hh..hh+3
                # and hh+4..hh+7): 9 bf16 main taps, 1 bf16 lo tap (2,2)
                # straight off a1v, 2 fp8 DoubleRow matmuls covering the
                # other 8 lo taps (4 each).
                movs = []
                for ty in range(3):
                    for tx in range(3):
                        movs.append(
                            (wt13[:, 3 * ty + tx, :], None,
                             lambda k, ty=ty, tx=tx: a1v[:, hh + ty + 4 * k : hh + ty + 4 * k + 4, tx : tx + W])
                        )
                movs.append(
                    (wsl2, None,
                     lambda k: a1v[0:64, hh + 2 + 4 * k : hh + 2 + 4 * k + 4, 2 : 2 + W])
                )
                movs.append(
                    (wa, mybir.MatmulPerfMode.DoubleRow,
                     lambda k: fa[:, :, hh + 4 * k : hh + 4 * k + 4, :])
                )
                movs.append(
                    (wb, mybir.MatmulPerfMode.DoubleRow,
                     lambda k: fb[:, :, hh + 4 * k : hh + 4 * k + 4, :])
                )
                nmov = len(movs)
                for wi, (wap, pm, mv) in enumerate(movs):
                    for k in range(2):
                        nc.tensor.matmul(
                            ps[:, k, :],
                            wap,
                            mv(k),
                            start=(wi == 0),
                            stop=(wi == nmov - 1),
                            perf_mode=pm,
                        )
                return ps

            def emit_pair_epi(p, ps, split_epi=False):
                psf = ps.rearrange("p a n -> p (a n)")
                slot = p % RING
                sq = epip.tile([128, NPIX], BF16, tag="sq")
                sgn = epip.tile([128, NPIX], F32, tag="sgn")
                v = epip.tile([128, NPIX], BF16, tag="v")
                v2 = epip.tile([128, NPIX], BF16, tag="v2")
                t3 = epip.tile([128, NPIX], F32, tag="t3")
                o = epip.tile([128, NPIX], F32, tag="o")

                def epi(c0, c1):
                    # scalar: s^2 (set 6 preloaded)
                    nc.scalar.activation(out=sq[:, c0:c1], in_=psf[:, c0:c1], func=AF.Square)
                    # DVE: sign via bitwise ops (single PSUM input)
                    nc.vector.tensor_scalar(
                        out=sgn.bitcast(U32)[:, c0:c1],
                        in0=psf.bitcast(U32)[:, c0:c1],
                        scalar1=0x80000000,
                        scalar2=0x3F800000,
                        op0=mybir.AluOpType.bitwise_and,
                        op1=mybir.AluOpType.bitwise_or,
                    )
                    nc.scalar.activation(out=v[:, c0:c1], in_=sq[:, c0:c1], func=AF.Ln)
                    nc.vector.tensor_tensor(
                        out=v2[:, c0:c1],
                        in0=v[:, c0:c1],
                        in1=lbc[:, slot * NPIX + c0 : slot * NPIX + c1],
                        op=mybir.AluOpType.subtract,
                    )
                    nc.scalar.activation(
                        out=t3[:, c0:c1], in_=v2[:, c0:c1], func=AF.Exp, scale=a2[:, :]
                    )
                    nc.vector.tensor_tensor(
                        out=o[:, c0:c1], in0=t3[:, c0:c1], in1=sgn[:, c0:c1],
                        op=mybir.AluOpType.mult,
                    )
                    nc.scalar.dma_start(
                        out=out_d[:, p * NPIX + c0 : p * NPIX + c1], in_=o[:, c0:c1]
                    )

                if split_epi:
                    epi(0, 512)
                    epi(512, NPIX)
                else:
                    epi(0, NPIX)

            def emit_pair(p, split_epi=False):
                emit_pair_epi(p, emit_pair_gemm(p), split_epi)

            # ---- prologue: s2/box two+ pairs ahead of the GEMM ----
            s2_slab(0)
            s2_slab(1)
            for p in (0, 1, 2):
                box_pair(p)

            # ---- fused streaming loop ----
            for s in range(NSLAB):
                issue_dma(s + 3)
                if s + 2 < NSLAB:
                    s2_slab(s + 2)
                for p in (2 * s + 3, 2 * s + 4):
                    if p < NPAIR:
                        box_pair(p)
                if s > 0:
                    emit_pair(2 * s - 1)
                emit_pair(2 * s)
            emit_pair(NPAIR - 1, split_epi=True)

    nc.compile()
    return nc


TAPS = [(ty, tx) for ty in range(3) for tx in range(3)]


def _host_prep(w, p, q):
    EPS = 1e-12
    w64 = w[0].astype(np.float64)  # (576, 128)
    qv = (q.astype(np.float64) ** 2 / 10.0)[0]
    wn = np.sqrt(np.maximum((w64**2).sum(0), EPS)) + qv
    what = (w64 / wn).astype(np.float32)
    wh = what.astype(ml_dtypes.bfloat16)
    wl = (what - wh.astype(np.float32)).astype(ml_dtypes.bfloat16).astype(np.float32)

    def tap(a, k):
        return np.ascontiguousarray(a[k * 64 : (k + 1) * 64, :])

    whs = (wh.astype(np.float32) * SC).astype(ml_dtypes.bfloat16)  # exact shift
    wt13 = np.stack([np.vstack([tap(whs, k), tap(whs, k)]) for k in range(9)])
    wls = wl * SC
    wsl2 = tap(wls, 8).astype(ml_dtypes.bfloat16)
    f8 = ml_dtypes.float8_e4m3
    # wa: partitions 0-63 pair (tap0, tap1); 64-127 pair (tap2, tap3)
    wa = np.stack([np.stack([tap(wls, 0), tap(wls, 1)], 1),
                   np.stack([tap(wls, 2), tap(wls, 3)], 1)]).reshape(128, 2, 128)
    wb = np.stack([np.stack([tap(wls, 4), tap(wls, 5)], 1),
                   np.stack([tap(wls, 6), tap(wls, 7)], 1)]).reshape(128, 2, 128)
    wa = wa.astype(f8)
    wb = wb.astype(f8)

    pe = np.exp(p.astype(np.float64) - p.astype(np.float64).max())
    a = pe / pe.sum()
    a2 = (a * 0.5).astype(np.float32).reshape(128, 1)

    band = np.zeros((128, 128), dtype=np.float32)
    for i in range(128):
        band[i, max(0, i - 1) : i + 2] = 1.0
    band = band.astype(ml_dtypes.bfloat16)
    return wt13, wsl2, wa, wb, a2, band


def _shift(x, dy, dx):
    """out[r, w, :] = x[r+dy, w+dx, :], zero outside."""
    out = np.zeros_like(x)
    r0, r1 = max(0, -dy), min(H, H - dy)
    c0, c1 = max(0, -dx), min(W, W - dx)
    out[r0:r1, c0:c1] = x[r0 + dy : r1 + dy, c0 + dx : c1 + dx]
    return out


def _host_images(xf):
    """xf: (H, W, C) fp32 -> img1 bf16, fa/fb fp8 plane-pairs, xs2 bf16."""
    xh = xf.astype(ml_dtypes.bfloat16)
    xl = (xf - xh.astype(np.float32)).astype(ml_dtypes.bfloat16)
    ph = np.zeros((HP, WP, C), dtype=ml_dtypes.bfloat16)
    pl = np.zeros((HP, WP, C), dtype=ml_dtypes.bfloat16)
    ph[1 : H + 1, 1 : W + 1] = xh
    pl[1 : H + 1, 1 : W + 1] = xl

    def T(img):  # (HP, WP, C) -> (C, HP*WP)
        return np.ascontiguousarray(img.transpose(2, 0, 1)).reshape(C, HP * WP)

    img1 = np.concatenate([T(ph), T(pl)], axis=0)

    f8 = ml_dtypes.float8_e4m3
    xh8 = xh.astype(f8)  # (H, W, C)

    def plane(k):  # tap k -> (C, H, W) fp8 shifted image
        ty, tx = TAPS[k]
        return np.ascontiguousarray(_shift(xh8, ty - 1, tx - 1).transpose(2, 0, 1))

    def pack(k00, k01, k10, k11):
        # [128 partitions, 2 planes, H, W]: p<64 pair (k00,k01); p>=64 (k10,k11)
        up = np.stack([plane(k00), plane(k01)], axis=1)   # (64, 2, H, W)
        lo = np.stack([plane(k10), plane(k11)], axis=1)
        return np.concatenate([up, lo], axis=0).reshape(128, 2 * H * W)

    fa = pack(0, 1, 2, 3)
    fb = pack(4, 5, 6, 7)
    xs2 = np.ascontiguousarray(xh.transpose(1, 0, 2))  # (W, H, C)
    return img1, fa, fb, xs2


LAST_RESULTS = None


def kernel(inputs, w, p, q):
    global LAST_RESULTS
    if "nc" not in _CACHE:
        _CACHE["nc"] = _build()
    nc = _CACHE["nc"]

    wt13, wsl2, wa, wb, a2, band = _host_prep(w, p, q)
    xf = np.asarray(inputs, dtype=np.float32)
    in_maps = []
    for b in range(B):
        img1, fa, fb, xs2 = _host_images(xf[b])
        in_maps.append(
            {
                "img1": img1,
                "fa": fa,
                "fb": fb,
                "xs2": xs2,
                "wt13": wt13,
                "wsl2": wsl2,
                "wa": wa,
                "wb": wb,
                "a2": a2,
                "band": band,
            }
        )
    import os

    trace = bool(int(os.environ.get("KERNEL_TRACE", "0")))
    res = run_bass_kernel_spmd(nc, in_maps, core_ids=list(range(B)), trace=trace)
    LAST_RESULTS = res
    out = np.stack(
        [res.results[b]["out"].T.reshape(H, W, UNITS) for b in range(B)]
    ).astype(np.float32)
    return out
